# revision 16
# baseline (speedup 1.0000x reference)
"""AttentionBlock (GroupNorm+SiLU -> qkv -> 8-head attn -> proj -> residual)
on 8 TRN2 NeuronCores, head-parallel.

The torch-faithful reshape q.transpose(1,2).reshape(B*NH,N,d) makes head h
attend over spatial block [512h, 512h+512) only (block-diagonal), so each
core computes the full pipeline for its 512 spatial positions with zero
collectives (GroupNorm stats recomputed per-core from a bf16 copy of x).

v2 speedups over the 296us baseline (trace-driven):
- row-tiled S matmuls (contraction d=64 -> two concurrent 64x128 PE tiles)
- O matmul in fp8e4m3 with DoubleRow perf mode (contraction-256 m-block
  pairs at 0.5 cyc/row); near-uniform softmax makes fp8 P/V noise average
  out (numpy-validated: 1.2e-4 rel err)
- exp split between ScalarE (exact) and VectorE (Schraudolph int8 bit
  trick, numpy-validated 6e-5 impact)
- all matmul operands bf16/fp8 -> FWL weight loads, dense PE stream that
  holds HAM K=8/8 (2.4 GHz); baseline idled into the 1.2 GHz throttle
- reciprocals batched across partitions ([8,512] once vs 8x [1,512])
"""

import sys

if "/opt/trn_rl_repo" not in sys.path:
    sys.path.append("/opt/trn_rl_repo")  # fallback; the axon-site copy wins

import math

import numpy as np

import concourse.bacc as bacc
import concourse.tile as tile
from concourse import mybir
from concourse.bass_utils import run_bass_kernel_spmd

F32 = mybir.dt.float32
F32R = mybir.dt.float32r
BF16 = mybir.dt.bfloat16
FP8 = mybir.dt.float8e4
I8 = mybir.dt.int8
I16 = mybir.dt.int16
AF = mybir.ActivationFunctionType
ALU = mybir.AluOpType
DR = mybir.MatmulPerfMode.DoubleRow

CH = 512          # channels
N = 4096          # spatial positions (64*64)
NB = 512          # spatial block per core
NCORES = 8
G = 32            # groups
GS = 16           # channels per group
EPS = 1e-5
SCALE = 0.125     # d ** -0.5, d = 64

USE_FP8_O = True      # DoubleRow fp8 O matmul; False -> bf16 [65,512] MMs
DVE_EXP = True        # Schraudolph exp on VectorE for half the tiles

# Schraudolph constants (exp(SCALE*s) -> target-format bits via affine+round)
LOG2E = 1.4426950408889634
A8 = SCALE * 8.0 * LOG2E            # fp8e4m3: 3 mantissa bits, bias 7
B8 = 7.0 * 8.0 - 0.35
A16 = SCALE * 128.0 * LOG2E         # bf16: 7 mantissa bits, bias 127
B16 = 127.0 * 128.0 - 5.59


def _build():
    nc = bacc.Bacc(None, target_bir_lowering=False)

    P = {}
    P["xfull_bf"] = nc.declare_dram_parameter("xfull_bf", [CH, N], BF16,
                                              isOutput=False)
    P["xblk"] = nc.declare_dram_parameter("xblk", [CH, NB], F32, isOutput=False)
    P["qkvwT"] = nc.declare_dram_parameter("qkvwT", [CH, 3 * CH], BF16,
                                           isOutput=False)
    P["qb"] = nc.declare_dram_parameter("qb", [128, 12], F32, isOutput=False)
    P["pwT2"] = nc.declare_dram_parameter("pwT2", [128, 2048], BF16,
                                          isOutput=False)
    P["pb"] = nc.declare_dram_parameter("pb", [128, 4], F32, isOutput=False)
    P["nw"] = nc.declare_dram_parameter("nw", [128, 4], F32, isOutput=False)
    P["nbias"] = nc.declare_dram_parameter("nbias", [128, 4], F32,
                                           isOutput=False)
    P["identB"] = nc.declare_dram_parameter("identB", [128, 128], BF16,
                                            isOutput=False)
    P["ones64"] = nc.declare_dram_parameter("ones64", [1, 64], F32R,
                                            isOutput=False)
    P["sel8"] = nc.declare_dram_parameter("sel8", [128, 8], F32, isOutput=False)
    P["selT"] = nc.declare_dram_parameter("selT", [8, 128], F32, isOutput=False)
    P["out"] = nc.declare_dram_parameter("out", [CH, NB], F32, isOutput=True)

    with tile.TileContext(nc) as tc:
        _emit(nc, tc, P)
    nc.finalize()
    return nc


def _emit(nc, tc, P):
    from contextlib import ExitStack

    with ExitStack() as es:
        # ---------------- persistent SBUF ----------------
        persist = es.enter_context(tc.tile_pool(name="persist", bufs=1))
        consts = es.enter_context(tc.tile_pool(name="consts", bufs=1))

        QT2 = persist.tile([128, N], BF16)        # [0:64]=Q^T, [64:128]=dup
        KT2 = persist.tile([128, 2048], BF16)     # [0:64]=m-blk 0-15, hi=16-31
        ONorm2 = persist.tile([128, N], BF16)     # [0:64]=attn out, hi=dup
        OuA = persist.tile([65, 8, 512], F32)     # unnorm O + denom row
        rD8 = persist.tile([4, 512], F32R)        # batched 1/D (spread)
        rD8b = persist.tile([4, 512], F32R)       # second batch
        rB = persist.tile([1, 8, 512], F32R)      # 1/D on partition 0
        xblk_sb = persist.tile([128, 4 * NB], F32)
        xblk_pb = persist.tile([128, 4 * NB], F32R)  # xblk + proj bias
        hsb = persist.tile([128, 2048], BF16)     # SiLU(norm(x)) own block
        qkvw_sb = persist.tile([128, 4 * 1536], BF16)
        pwT2_sb = persist.tile([128, 2048], BF16)
        vs = persist.tile([128, 2048], BF16)
        if USE_FP8_O:
            Vp = persist.tile([128, 16, 2, 80], FP8)
        else:
            Vp = persist.tile([128, 16, 2, 65], BF16)

        pbt_pool = es.enter_context(tc.tile_pool(name="pbt", bufs=2))

        qb_sb = consts.tile([128, 12], F32)
        pb_sb = consts.tile([128, 4], F32)
        nw_sb = consts.tile([128, 4], F32)
        nb_sb = consts.tile([128, 4], F32)
        identB_sb = consts.tile([128, 128], BF16)
        ones64_sb = consts.tile([1, 64], F32R)
        sel8_sb = consts.tile([128, 8], F32)
        selT_sb = consts.tile([8, 128], F32)
        eps_sb = consts.tile([128, 1], F32)
        A_sb = consts.tile([128, 4], F32)
        B_sb = consts.tile([128, 4], F32)

        for t in range(4):
            nc.sync.dma_start(out=xblk_sb[:, t * NB:(t + 1) * NB],
                              in_=P["xblk"][t * 128:(t + 1) * 128, :])
        nc.sync.dma_start(out=qb_sb[:], in_=P["qb"][:])
        nc.sync.dma_start(out=pb_sb[:], in_=P["pb"][:])
        nc.sync.dma_start(out=nw_sb[:], in_=P["nw"][:])
        nc.sync.dma_start(out=nb_sb[:], in_=P["nbias"][:])
        nc.sync.dma_start(out=identB_sb[:], in_=P["identB"][:])
        nc.sync.dma_start(out=ones64_sb[:], in_=P["ones64"][:])
        nc.sync.dma_start(out=sel8_sb[:], in_=P["sel8"][:])
        nc.sync.dma_start(out=selT_sb[:], in_=P["selT"][:])
        for kt in range(4):
            nc.sync.dma_start(out=qkvw_sb[:, kt * 1536:(kt + 1) * 1536],
                              in_=P["qkvwT"][kt * 128:(kt + 1) * 128, :])
        nc.sync.dma_start(out=pwT2_sb[:], in_=P["pwT2"][:])
        nc.vector.memset(eps_sb[:], EPS)

        # ---------- phase B: GroupNorm stats from bf16 full x ----------
        with ExitStack() as es_b:
            pools = es_b.enter_context(tc.tile_pool(name="pools", bufs=4))
            psA = es_b.enter_context(tc.tile_pool(name="psA", bufs=1,
                                                  space="PSUM"))

            chs = pools.tile([128, 8], F32, tag="chs")
            scr = pools.tile([128, N], BF16, tag="scr")  # ACT dummy out
            for t in range(4):
                xf = pools.tile([128, N], BF16, tag="xf", bufs=2)
                nc.sync.dma_start(out=xf[:],
                                  in_=P["xfull_bf"][t * 128:(t + 1) * 128, :])
                if t >= 2:
                    # ScalarE path: sums via activation accumulators
                    ac1 = pools.tile([128, 1], F32, tag="ac1")
                    ac2 = pools.tile([128, 1], F32, tag="ac2")
                    nc.scalar.activation(out=scr[:], in_=xf[:],
                                         func=AF.Identity, accum_out=ac1[:])
                    nc.scalar.activation(out=scr[:], in_=xf[:],
                                         func=AF.Square, accum_out=ac2[:])
                    nc.vector.tensor_scalar(
                        out=chs[:, 2 * t:2 * t + 1], in0=ac1[:],
                        scalar1=1.0 / N, scalar2=None, op0=ALU.mult)
                    nc.vector.tensor_scalar(
                        out=chs[:, 2 * t + 1:2 * t + 2], in0=ac2[:],
                        scalar1=1.0 / N, scalar2=None, op0=ALU.mult)
                    continue
                st = pools.tile([128, 8, 6], F32, tag="st")
                for k in range(8):
                    nc.vector.bn_stats(out=st[:, k, :],
                                       in_=xf[:, k * 512:(k + 1) * 512])
                mv = pools.tile([128, 2], F32, tag="mv")
                nc.vector.bn_aggr(out=mv[:], in_=st[:])
                nc.vector.tensor_copy(chs[:, 2 * t:2 * t + 1], mv[:, 0:1])
                nc.vector.tensor_scalar(
                    out=chs[:, 2 * t + 1:2 * t + 2], in0=mv[:, 0:1],
                    scalar1=mv[:, 0:1], scalar2=None, op0=ALU.mult)
                nc.vector.tensor_tensor(
                    out=chs[:, 2 * t + 1:2 * t + 2],
                    in0=chs[:, 2 * t + 1:2 * t + 2], in1=mv[:, 1:2],
                    op=ALU.add)

            gp = psA.tile([8, 8], F32, tag="gp")
            for t in range(4):
                nc.tensor.matmul(gp[:, 2 * t:2 * t + 2], lhsT=sel8_sb[:],
                                 rhs=chs[:, 2 * t:2 * t + 2],
                                 start=True, stop=True)
            gar_sb = pools.tile([8, 8], F32, tag="gar")
            nc.vector.tensor_copy(gar_sb[:], gp[:])
            gx = psA.tile([128, 8], F32, tag="gx")
            for t in range(4):
                nc.tensor.matmul(gx[:, 2 * t:2 * t + 2], lhsT=selT_sb[:],
                                 rhs=gar_sb[:, 2 * t:2 * t + 2],
                                 start=True, stop=True)
            gxs = pools.tile([128, 8], F32, tag="gxs")
            nc.vector.tensor_copy(gxs[:], gx[:])
            gx3 = gxs.rearrange("p (t two) -> p t two", two=2)
            musq = pools.tile([128, 4], F32, tag="musq")
            nc.vector.tensor_tensor(out=musq[:], in0=gx3[:, :, 0],
                                    in1=gx3[:, :, 0], op=ALU.mult)
            var = pools.tile([128, 4], F32, tag="var")
            nc.vector.tensor_tensor(out=var[:], in0=gx3[:, :, 1], in1=musq[:],
                                    op=ALU.subtract)
            sd = pools.tile([128, 4], F32, tag="sd")
            nc.scalar.activation(out=sd[:], in_=var[:], func=AF.Sqrt,
                                 bias=eps_sb[:], scale=1.0)
            rstd = pools.tile([128, 4], F32, tag="rstd")
            nc.vector.reciprocal(out=rstd[:], in_=sd[:])
            nc.vector.tensor_tensor(out=A_sb[:], in0=rstd[:], in1=nw_sb[:],
                                    op=ALU.mult)
            muA = pools.tile([128, 4], F32, tag="muA")
            nc.vector.tensor_tensor(out=muA[:], in0=gx3[:, :, 0], in1=A_sb[:],
                                    op=ALU.mult)
            nc.vector.tensor_tensor(out=B_sb[:], in0=nb_sb[:], in1=muA[:],
                                    op=ALU.subtract)

            # xblk + proj-bias precompute (for the fused epilogue add)
            for t in range(4):
                nc.vector.tensor_scalar(
                    out=xblk_pb[:, t * 512:(t + 1) * 512],
                    in0=xblk_sb[:, t * 512:(t + 1) * 512],
                    scalar1=pb_sb[:, t:t + 1], scalar2=None, op0=ALU.add)

        # ---------- phase C: SiLU + qkv + layouts ----------
        with ExitStack() as es_c:
            poolq = es_c.enter_context(tc.tile_pool(name="poolq", bufs=1))
            psB = es_c.enter_context(tc.tile_pool(name="psB", bufs=3,
                                                  space="PSUM"))
            psT = es_c.enter_context(tc.tile_pool(name="psT", bufs=2,
                                                  space="PSUM"))

            for t in range(4):
                nc.scalar.activation(out=hsb[:, t * 512:(t + 1) * 512],
                                     in_=xblk_sb[:, t * 512:(t + 1) * 512],
                                     func=AF.Silu,
                                     bias=B_sb[:, t:t + 1],
                                     scale=A_sb[:, t:t + 1])

            stage_q = poolq.tile([128, 2048], BF16)
            stage_k = poolq.tile([128, 2048], BF16)
            for ot in range(12):
                ps = psB.tile([128, 512], F32, tag="qkvps")
                for kt in range(4):
                    nc.tensor.matmul(
                        ps[:],
                        lhsT=qkvw_sb[:, kt * 1536 + ot * 128:
                                     kt * 1536 + (ot + 1) * 128],
                        rhs=hsb[:, kt * 512:(kt + 1) * 512],
                        start=(kt == 0), stop=(kt == 3))
                kind, t = ot // 4, ot % 4
                if kind == 0:
                    # Q: chunks 2t (rows 0:64) even, 2t+1 (rows 64:128) odd
                    nc.scalar.activation(
                        out=QT2[0:64, (2 * t) * 512:(2 * t + 1) * 512],
                        in_=ps[0:64, :], func=AF.Identity,
                        bias=qb_sb[0:64, ot:ot + 1])
                    nc.scalar.activation(
                        out=stage_q[64:128, t * 512:(t + 1) * 512],
                        in_=ps[64:128, :], func=AF.Identity,
                        bias=qb_sb[64:128, ot:ot + 1])
                    nc.sync.dma_start(
                        out=QT2[0:64, (2 * t + 1) * 512:(2 * t + 2) * 512],
                        in_=stage_q[64:128, t * 512:(t + 1) * 512])
                elif kind == 1:
                    # K: chunk c -> m-blocks 4c..4c+3; c<4 -> KT2 lo, else hi
                    if t < 2:
                        nc.scalar.activation(
                            out=KT2[0:64, (2 * t) * 512:(2 * t + 1) * 512],
                            in_=ps[0:64, :], func=AF.Identity,
                            bias=qb_sb[0:64, ot:ot + 1])
                        nc.scalar.activation(
                            out=stage_k[64:128, t * 512:(t + 1) * 512],
                            in_=ps[64:128, :], func=AF.Identity,
                            bias=qb_sb[64:128, ot:ot + 1])
                        nc.sync.dma_start(
                            out=KT2[0:64, (2 * t + 1) * 512:(2 * t + 2) * 512],
                            in_=stage_k[64:128, t * 512:(t + 1) * 512])
                    else:
                        tt = t - 2
                        nc.vector.tensor_scalar(
                            out=stage_k[0:64, t * 512:(t + 1) * 512],
                            in0=ps[0:64, :], scalar1=qb_sb[0:64, ot:ot + 1],
                            scalar2=None, op0=ALU.add)
                        nc.sync.dma_start(
                            out=KT2[64:128, (2 * tt) * 512:(2 * tt + 1) * 512],
                            in_=stage_k[0:64, t * 512:(t + 1) * 512])
                        nc.vector.tensor_scalar(
                            out=KT2[64:128,
                                    (2 * tt + 1) * 512:(2 * tt + 2) * 512],
                            in0=ps[64:128, :],
                            scalar1=qb_sb[64:128, ot:ot + 1],
                            scalar2=None, op0=ALU.add)
                else:
                    # V
                    nc.vector.tensor_scalar(
                        out=vs[:, t * 512:(t + 1) * 512], in0=ps[:],
                        scalar1=qb_sb[:, ot:ot + 1], scalar2=None, op0=ALU.add)

            # duplicate Q^T across partition halves for row tiling
            nc.sync.dma_start(out=QT2[64:128, :], in_=QT2[0:64, :])

            # V layout: PE transposes -> Vp[m, pair, half, d] (+ones col)
            nc.vector.memset(Vp[:, :, :, 64:65], 1.0)
            for tt in range(4):
                for b in range(4):
                    pst = psT.tile([128, 128], BF16, tag="vtr")
                    nc.tensor.transpose(
                        pst[:],
                        in_=vs[:, tt * 512 + b * 128:tt * 512 + (b + 1) * 128],
                        identity=identB_sb[:])
                    j1, j2 = 8 * tt + b, 8 * tt + 4 + b
                    nc.vector.tensor_copy(Vp[:, j1 % 16, j1 // 16, 0:64],
                                          pst[:, 0:64])
                    nc.vector.tensor_copy(Vp[:, j2 % 16, j2 // 16, 0:64],
                                          pst[:, 64:128])

        # ---------- phase E: attention (S row-tiled | exp split | O DR) ----
        with ExitStack() as es_e:
            psS = es_e.enter_context(tc.tile_pool(name="psS", bufs=3,
                                                  space="PSUM"))
            psO = es_e.enter_context(tc.tile_pool(name="psO", bufs=1,
                                                  space="PSUM"))
            psD = es_e.enter_context(tc.tile_pool(name="psD", bufs=1,
                                                  space="PSUM"))
            poolsm = es_e.enter_context(tc.tile_pool(name="poolsm", bufs=2))

            PBts = {}
            psOt = {}

            def emit_S_phase(I):
                isl = slice(I * 512, (I + 1) * 512)
                PBts[I] = pbt_pool.tile(
                    [128, 16, 1024], FP8 if USE_FP8_O else BF16,
                    tag="PBt", name=f"PBt{I}")
                for p in range(16):
                    sp = psS.tile([128, 1024], F32, tag="sp")
                    nc.tensor.matmul(sp[:, 0:512],
                                     lhsT=KT2[0:64, p * 128:(p + 1) * 128],
                                     rhs=QT2[0:64, isl],
                                     start=True, stop=True)
                    nc.tensor.matmul(sp[:, 512:1024],
                                     lhsT=KT2[64:128, p * 128:(p + 1) * 128],
                                     rhs=QT2[64:128, isl],
                                     start=True, stop=True)
                    if DVE_EXP and p in (1, 3, 5, 7, 9, 11, 13):
                        nc.vector.tensor_scalar(
                            out=PBts[I][:, p, :].bitcast(
                                I8 if USE_FP8_O else I16),
                            in0=sp[:],
                            scalar1=(A8 if USE_FP8_O else A16),
                            scalar2=(B8 if USE_FP8_O else B16),
                            op0=ALU.mult, op1=ALU.add)
                    else:
                        nc.scalar.activation(out=PBts[I][:, p, :], in_=sp[:],
                                             func=AF.Exp, scale=SCALE)

            def emit_O_phase(I):
                ops = psO.tile([65, 512], F32, tag="ops")
                psOt[I] = ops
                pb4 = PBts[I].rearrange("p s (two n) -> p s two n", two=2)
                if USE_FP8_O:
                    for p in range(16):
                        nc.tensor.matmul(ops[:], lhsT=Vp[:, p, :, 0:65],
                                         rhs=pb4[:, p, :, :],
                                         start=(p == 0), stop=(p == 15),
                                         perf_mode=DR)
                else:
                    for p in range(16):
                        for ki in range(2):
                            nc.tensor.matmul(
                                ops[:], lhsT=Vp[:, p, ki, 0:65],
                                rhs=pb4[:, p, ki, :],
                                start=(p == 0 and ki == 0),
                                stop=(p == 15 and ki == 1))
                nc.vector.tensor_copy(OuA[:, I, :], ops[:])
                del PBts[I], psOt[I]

            def emit_norm(i):
                # dps = broadcast of 1/D over 64 partitions, then multiply
                dps = psD.tile([64, 512], F32, tag="dps")
                nc.tensor.matmul(dps[:], lhsT=ones64_sb[:],
                                 rhs=rB[0:1, i, :], start=True, stop=True)
                nc.vector.tensor_tensor(
                    out=ONorm2[0:64, i * 512:(i + 1) * 512],
                    in0=OuA[0:64, i, :], in1=dps[:], op=ALU.mult)
                nc.sync.dma_start(
                    out=ONorm2[64:128, i * 512:(i + 1) * 512],
                    in_=ONorm2[0:64, i * 512:(i + 1) * 512])

            for I in range(8):
                emit_S_phase(I)
                if I > 0:
                    emit_O_phase(I - 1)
                if I == 6:
                    # denominators of I0..3 -> partitions 0..3, one recip,
                    # then bounce to partition 0 for the expand matmuls
                    nc.sync.dma_start(out=rD8[0:4, :].bitcast(F32),
                                      in_=OuA[64:65, 0:4, :])
                    with nc.allow_low_precision(reason="f32r==f32 bits"):
                        nc.vector.reciprocal(out=rD8[0:4, :],
                                             in_=rD8[0:4, :])
                    nc.sync.dma_start(out=rB[0:1, 0:4, :], in_=rD8[0:4, :])
                if I == 7:
                    for i in range(4):
                        emit_norm(i)
            emit_O_phase(7)
            nc.sync.dma_start(out=rD8b[0:4, :].bitcast(F32),
                              in_=OuA[64:65, 4:8, :])
            with nc.allow_low_precision(reason="f32r==f32 bits"):
                nc.vector.reciprocal(out=rD8b[0:4, :], in_=rD8b[0:4, :])
            nc.sync.dma_start(out=rB[0:1, 4:8, :], in_=rD8b[0:4, :])

            # proj lo-half (chunks 0-3) for ot 0-2 first: only needs
            # norms 0-3, so the PE works through it while the second
            # reciprocal batch resolves. Only 3 pp tiles may be in flight
            # (psS pool bufs=3) or the PE queue head-of-line deadlocks.
            pps = {}

            def emit_ppA(ot):
                pp = psS.tile([128, 1024], F32, tag="sp", name=f"pp{ot}")
                pps[ot] = pp
                for c in range(4):
                    nc.tensor.matmul(
                        pp[:, 0:512],
                        lhsT=pwT2_sb[0:64, c * 512 + ot * 128:
                                     c * 512 + (ot + 1) * 128],
                        rhs=ONorm2[0:64, c * 512:(c + 1) * 512],
                        start=(c == 0), stop=(c == 3))

            def emit_ppB_fin(ot):
                pp = pps[ot]
                for c in range(4):
                    nc.tensor.matmul(
                        pp[:, 512:1024],
                        lhsT=pwT2_sb[64:128, c * 512 + ot * 128:
                                     c * 512 + (ot + 1) * 128],
                        rhs=ONorm2[64:128, (c + 4) * 512:(c + 5) * 512],
                        start=(c == 0), stop=(c == 3))
                fin = poolsm.tile([128, 512], F32, tag="fin")
                nc.vector.tensor_tensor(
                    out=fin[:],
                    in0=xblk_pb[:, ot * 512:(ot + 1) * 512].bitcast(F32),
                    in1=pp[:, 0:512], op=ALU.add)
                nc.vector.tensor_tensor(out=fin[:], in0=fin[:],
                                        in1=pp[:, 512:1024], op=ALU.add)
                nc.sync.dma_start(out=P["out"][ot * 128:(ot + 1) * 128, :],
                                  in_=fin[:])
                del pps[ot]

            for ot in range(3):
                emit_ppA(ot)
            for i in range(4, 8):
                emit_norm(i)
            emit_ppB_fin(0)
            emit_ppA(3)
            for ot in range(1, 4):
                emit_ppB_fin(ot)


def _host_inputs(x, norm_w, norm_b, qkv_w, qkv_b, proj_w, proj_b):
    import ml_dtypes
    x2d = np.ascontiguousarray(np.asarray(x, np.float32).reshape(CH, N))
    qkv_w = np.asarray(qkv_w, np.float32)
    proj_w = np.asarray(proj_w, np.float32)
    pwT64 = proj_w.T.reshape(8, 64, CH).transpose(1, 0, 2)  # [64, 8, 512]
    pwT2 = np.concatenate([
        np.ascontiguousarray(pwT64[:, 0:4, :]).reshape(64, 2048),
        np.ascontiguousarray(pwT64[:, 4:8, :]).reshape(64, 2048)], axis=0)
    common = {
        "xfull_bf": np.ascontiguousarray(x2d).astype(ml_dtypes.bfloat16),
        "qkvwT": np.ascontiguousarray(qkv_w.T).astype(ml_dtypes.bfloat16),
        "qb": np.ascontiguousarray(
            np.asarray(qkv_b, np.float32).reshape(12, 128).T),
        "pwT2": np.ascontiguousarray(pwT2).astype(ml_dtypes.bfloat16),
        "pb": np.ascontiguousarray(
            np.asarray(proj_b, np.float32).reshape(4, 128).T),
        "nw": np.ascontiguousarray(
            np.asarray(norm_w, np.float32).reshape(4, 128).T),
        "nbias": np.ascontiguousarray(
            np.asarray(norm_b, np.float32).reshape(4, 128).T),
        "identB": np.eye(128, dtype=np.float32).astype(ml_dtypes.bfloat16),
        "ones64": np.ones((1, 64), np.float32),
        "sel8": np.ascontiguousarray(
            (np.arange(128)[:, None] // GS == np.arange(8)[None, :])
            .astype(np.float32) / GS),
        "selT": np.ascontiguousarray(
            (np.arange(128)[None, :] // GS == np.arange(8)[:, None])
            .astype(np.float32)),
    }
    in_maps = []
    for h in range(NCORES):
        m = dict(common)
        m["xblk"] = np.ascontiguousarray(x2d[:, h * NB:(h + 1) * NB])
        in_maps.append(m)
    return in_maps


_LAST_RESULT = {}


def kernel(x, norm_w, norm_b, qkv_w, qkv_b, proj_w, proj_b, _trace=False):
    nc = _build()
    in_maps = _host_inputs(x, norm_w, norm_b, qkv_w, qkv_b, proj_w, proj_b)
    res = run_bass_kernel_spmd(nc, in_maps, core_ids=list(range(NCORES)),
                               trace=_trace)
    _LAST_RESULT["res"] = res
    full = np.concatenate([res.results[h]["out"] for h in range(NCORES)],
                          axis=1)
    return full.reshape(1, CH, 64, 64).astype(np.float32)
